# revision 16
# baseline (speedup 1.0000x reference)
"""Multi-head attention (B=2, N=2048, C=1024, H=16, D=64) on 8 TRN2 cores.

Sharding: tensor-parallel over heads — 2 heads per core. Each core computes
Q/K/V projections for its 2 heads, attention, and a partial output
projection (its heads' slice of Wo). Host sums the 8 partial outputs + bo.

v2 structure (vs v1): the attention inner loop is a conveyor paced by
ScalarE exp (~1147ns per key tile). ctx matmuls trail their exp by one
2-key-tile group so the PE FIFO never waits on ACT; projection chains,
V transposes and out-projection matmuls are spread as single-op fillers
between groups instead of bursts. x streams in token-major [128,512]
tiles so the first K/V chain starts ~3us in. Normalize reads ctx straight
from PSUM (no staging copies); out-proj results are copied to bf16 and
DMAd as bf16 partials (host accumulates in fp32).

Per-core dataflow (all matmul inputs bf16, PSUM accumulation fp32):
  xT [1024, 4096] (x transposed on host, replicated to all cores)
  QT/KT = W.T @ x.T   -> [128 (2 heads x 64), 4096]  (lhsT=W chunk, rhs=xT)
  VT likewise, then PE-transposed into v_aug [keys, 65] per head
  (65th column = ones -> softmax denominator comes out of the ctx matmul)
  S^T = K @ Q.T  -> [keys, q] in PSUM (row-tiled: both heads concurrent);
  exp on ScalarE -> bf16 SBUF
  ctx^T_aug [65, q] = v_aug.T @ expS^T  (row 64 = denominator)
  normalize: recip(row 64) on DVE, gpsimd partition_broadcast, DVE multiply
  out_partial [4096, 1024] = ctx^T.T @ Wo_slice  (bf16 out, summed on host)

The 1/sqrt(D) scale is folded into Wq/bq on the host (exact: 0.125).
"""

import numpy as np
import ml_dtypes

import concourse.bass as bass
from concourse import bacc
import concourse.tile as tile
from concourse import mybir, library_config
from concourse.bass_utils import run_bass_kernel_spmd

BF16 = mybir.dt.bfloat16
F32 = mybir.dt.float32
F8 = mybir.dt.float8e4

B, N, C = 2, 2048, 1024
H, D = 16, 64
T = B * N              # 4096 tokens
HPC = H // 8           # heads per core = 2
DPC = HPC * D          # head dims per core = 128
KCH = C // 128         # 8 contraction chunks for projections
NCH = T // 512         # 8 token chunks of 512
KT16 = N // 128        # 16 key tiles per batch


def build_core_program(nc):
    # x pre-arranged on host to [128, token-chunk, k-chunk, 512] so each
    # 512-token chunk is ONE dma with 128 contiguous-8KB descriptors
    xT = nc.dram_tensor("xT", [128, NCH, KCH, 512], BF16,
                        kind="ExternalInput").ap()
    # weights pre-arranged on host to the SBUF layout [128, KCH, DPC] so
    # their DMAs are straight 2KB-row copies (the on-the-fly rearrange
    # generated 1024 tiny 256B descriptors and took ~4us per weight)
    wq = nc.dram_tensor("wq", [128, KCH, DPC], BF16,
                        kind="ExternalInput").ap()
    wk = nc.dram_tensor("wk", [128, KCH, DPC], BF16,
                        kind="ExternalInput").ap()
    wv = nc.dram_tensor("wv", [128, KCH, DPC], BF16,
                        kind="ExternalInput").ap()
    wo = nc.dram_tensor("wo", [DPC, C], BF16, kind="ExternalInput").ap()
    bqkv = nc.dram_tensor("bqkv", [DPC, 3], F32, kind="ExternalInput").ap()
    iden = nc.dram_tensor("iden", [128, 128], BF16, kind="ExternalInput").ap()
    out = nc.dram_tensor("out", [T, C], BF16, kind="ExternalOutput").ap()

    with tile.TileContext(nc) as tc:
        with tc.tile_pool(name="singles", bufs=1) as singles:
            # DMA trigger issue costs ~620ns each on an engine queue, so
            # trigger ORDER and COUNT dominate startup. Urgent data (token
            # chunk 0 + proj weights) goes first on Sync; the bulk of x
            # rides 8 big [128,3584] DMAs issued from the ScalarE queue
            # (idle until the exp conveyor starts). load_library is
            # emitted after all input DMAs — its ~6us IRAM load otherwise
            # blocks the first triggers.
            # warmup fodder (no DMA dependency): ramps the HAM clock gate
            # from t~0 while input DMAs are still in flight, and forces
            # the Exp ACT table load off the critical path
            warm = singles.tile([128, 256], BF16, tag="warm")
            nc.vector.memset(warm, 0.015625)
            edum = singles.tile([1, 8], F32, tag="edum")
            nc.vector.memset(edum, 0.5)
            edumo = singles.tile([1, 8], BF16, tag="edumo")

            # urgent inputs on the Sync queue: x chunks 0-1 + K/Q weights
            # get the full HBM bandwidth first. Bulk x (chunks 2-7) + wo
            # are issued from the GpSimd queue AFTER load_library — its
            # ~6us IRAM load naturally delays them past the urgent window.
            x_sb = singles.tile([128, NCH, KCH, 512], BF16, tag="xsb")
            for c in (0, 1):
                nc.sync.dma_start(out=x_sb[:, c], in_=xT[:, c])
            w_sb = {}
            for nm, w in (("wk", wk), ("wq", wq)):
                t = singles.tile([128, KCH, DPC], BF16, tag=nm, name=nm)
                nc.sync.dma_start(out=t, in_=w)
                w_sb[nm] = [t[:, k, :] for k in range(KCH)]
            bqkv_sb = singles.tile([DPC, 3], F32, tag="bqkv")
            nc.sync.dma_start(out=bqkv_sb, in_=bqkv)
            b_sb = {"q": bqkv_sb[:, 0:1], "k": bqkv_sb[:, 1:2],
                    "v": bqkv_sb[:, 2:3]}
            t = singles.tile([128, KCH, DPC], BF16, tag="wv", name="wv_sb")
            nc.sync.dma_start(out=t, in_=wv)
            w_sb["wv"] = [t[:, k, :] for k in range(KCH)]
            id_sb = singles.tile([128, 128], BF16, tag="iden")
            nc.sync.dma_start(out=id_sb, in_=iden)

            nc.gpsimd.load_library(library_config.proxy)
            for c in range(2, NCH):
                nc.gpsimd.dma_start(out=x_sb[:, c], in_=xT[:, c])
            wo_sb = singles.tile([DPC, C], BF16, tag="wo")
            nc.gpsimd.dma_start(out=wo_sb, in_=wo)

            def xslice(k, nch):
                return x_sb[:, nch, k, :]

            QT = singles.tile([128, T], BF16, tag="QT")
            KTt = singles.tile([128, T], BF16, tag="KT")
            VT = singles.tile([128, T], BF16, tag="VT")
            ctxTn = singles.tile([128, T], BF16, tag="ctxTn")
            # v_aug pairs for fp8 DoubleRow ctx: [128 keys, group, j, 80]
            # (j = which key tile of the pair; stride 80 keeps the DR
            # k-tile step 16B-aligned). col 64 = 16.0: Wv is host-scaled
            # x16 so v and the ones column carry the same factor and the
            # softmax normalization cancels it exactly.
            vaug = [[singles.tile([128, KT16 // 2, 2, 80], F8,
                                  tag=f"vaug{b}{h}", name=f"vaug{b}{h}")
                     for h in range(HPC)] for b in range(B)]
            for b in range(B):
                for h in range(HPC):
                    nc.vector.memset(vaug[b][h][:, :, :, D:D + 1], 16.0)

            with tc.tile_pool(name="psP", bufs=1, space="PSUM") as psP, \
                    tc.tile_pool(name="psO", bufs=1, space="PSUM") as psO, \
                    tc.tile_pool(name="psS", bufs=2, space="PSUM") as psS, \
                    tc.tile_pool(name="psC", bufs=1, space="PSUM") as psC, \
                    tc.tile_pool(name="esb", bufs=6) as esb, \
                    tc.tile_pool(name="nrm", bufs=3) as nrm, \
                    tc.tile_pool(name="osb", bufs=3) as osb:

                # HAM warmup on a memset tile — zero DMA dependency, so
                # the PE clock ramps from t~0 while x/weights stream in.
                # ~12x256 rows keeps PE continuously busy ~3us (the ramp
                # window). The dummy exp pulls the 1.3us Exp ACT_TABLE_LOAD
                # to t~0 instead of stalling the first real exp.
                nc.scalar.activation(edumo, edum,
                                     mybir.ActivationFunctionType.Exp)
                for wu in range(12):
                    pw = psO.tile([128, 256], F32, tag="po", name="pw")
                    nc.tensor.matmul(out=pw, lhsT=warm[:, 0:128],
                                     rhs=warm, start=True, stop=True)

                # ---- filler op factories (each closure emits ~1 PE op) --

                def chain_ops(nm, dstT, nch, act_bias=False):
                    """QKV projection chain: 8 accumulating matmuls into a
                    psP bank + one bias-add move to SBUF. Returns 9 ops."""
                    st = {}

                    def mk(k):
                        def op():
                            if k == 0:
                                st["ps"] = psP.tile([128, 512], F32,
                                                    tag="pj", name="pj")
                            nc.tensor.matmul(
                                out=st["ps"], lhsT=w_sb[nm][k],
                                rhs=xslice(k, nch),
                                start=(k == 0), stop=(k == KCH - 1))
                        return op

                    def mv():
                        dst = dstT[:, nch * 512:(nch + 1) * 512]
                        if act_bias:
                            nc.scalar.activation(
                                out=dst, in_=st["ps"],
                                func=mybir.ActivationFunctionType.Identity,
                                bias=b_sb[nm[1]], scale=1.0)
                        else:
                            nc.vector.tensor_scalar_add(
                                out=dst, in0=st["ps"], scalar1=b_sb[nm[1]])
                    return [mk(k) for k in range(KCH)] + [mv]

                def transpose_ops(nch):
                    """4 V transposes for token chunk nch -> vaug tiles."""
                    ops = []

                    def mk(t16):
                        def op():
                            b, bt = divmod(t16, KT16)
                            g, j = divmod(bt, 2)
                            pt = psO.tile([128, 128], BF16, tag="po",
                                          name="pt")
                            base = t16 * 128
                            nc.tensor.transpose(
                                pt, VT[:, base:base + 128], id_sb)
                            nc.vector.tensor_copy(
                                out=vaug[b][0][:, g, j, 0:D],
                                in_=pt[:, 0:D])
                            nc.vector.tensor_copy(
                                out=vaug[b][1][:, g, j, 0:D],
                                in_=pt[:, D:2 * D])
                        return op
                    for t16 in range(nch * 4, nch * 4 + 4):
                        ops.append(mk(t16))
                    return ops

                def outproj_ops(ch):
                    """8 out-proj matmuls for chunk ch (MM -> bf16 copy
                    into a [128,4,512] staging tile; one DMA per c-half =
                    2 Sync triggers per chunk)."""
                    q0 = ch * 512
                    st = {}
                    ops = []

                    def mk(j):
                        def op():
                            nch2, t4 = divmod(j, 4)
                            tok = q0 + t4 * 128
                            po = psO.tile([128, 512], F32, tag="po",
                                          name="po")
                            nc.tensor.matmul(
                                out=po, lhsT=ctxTn[:, tok:tok + 128],
                                rhs=wo_sb[:, nch2 * 512:(nch2 + 1) * 512],
                                start=True, stop=True)
                            if t4 == 0:
                                st[nch2] = osb.tile([128, 4, 512], BF16,
                                                    tag="ot", name="ot")
                            nc.vector.tensor_copy(out=st[nch2][:, t4, :],
                                                  in_=po)
                            if t4 == 3:
                                nc.sync.dma_start(
                                    out=out[q0:q0 + 512,
                                            nch2 * 512:(nch2 + 1) * 512
                                            ].rearrange(
                                                "(t p) c -> p t c", p=128),
                                    in_=st[nch2])
                        return op
                    for j in range(8):
                        ops.append(mk(j))
                    return ops

                # ---- attention pieces ----------------------------------

                def emit_scores_exp(b, qch, kc, eSg):
                    """Row-tiled packed scores (both heads concurrent) +
                    exp into half of the group's fp8 eS pair tile."""
                    q0 = b * N + qch * 512
                    k0 = b * N + kc * 128
                    pS = psS.tile([128, 1024], F32, tag="s", name="s")
                    for h in range(HPC):
                        nc.tensor.matmul(
                            out=pS[:, h * 512:(h + 1) * 512],
                            lhsT=KTt[h * D:(h + 1) * D, k0:k0 + 128],
                            rhs=QT[h * D:(h + 1) * D, q0:q0 + 512],
                            start=True, stop=True)
                    nc.scalar.activation(
                        eSg[:, kc % 2, :], pS,
                        mybir.ActivationFunctionType.Exp)

                def emit_ctx(b, g, eSg, ctx):
                    """One fp8 DoubleRow matmul per head contracts the
                    group's 2 key tiles (256 keys) at 0.5 cyc/row."""
                    for h in range(HPC):
                        nc.tensor.matmul(
                            out=ctx[h],
                            lhsT=vaug[b][h][:, g, :, 0:D + 1],
                            rhs=eSg[:, :, h * 512:(h + 1) * 512],
                            start=(g == 0), stop=(g == KT16 // 2 - 1),
                            perf_mode=mybir.MatmulPerfMode.DoubleRow)

                def emit_normalize(q0, ctx):
                    """denominator recip -> broadcast -> multiply into
                    ctxTn. dn/ctxs staged in SBUF (custom-DVE recip can't
                    read PSUM); the big multiply runs h0 on gpsimd, h1 on
                    DVE as in v1."""
                    bcs, ctxss = [], []
                    for h in range(HPC):
                        dn = nrm.tile([1, 512], F32, tag=f"dn{h}",
                                      name=f"dn{h}")
                        nc.vector.tensor_copy(dn, ctx[h][D:D + 1, :])
                        ctxs = nrm.tile([D, 512], F32, tag=f"ctxs{h}",
                                        name=f"ctxs{h}")
                        nc.vector.tensor_copy(ctxs, ctx[h][0:D, :])
                        rc = nrm.tile([1, 512], F32, tag=f"rc{h}",
                                      name=f"rc{h}")
                        nc.vector.reciprocal_approx_fast(rc, dn)
                        bc = nrm.tile([D, 512], F32, tag=f"bc{h}",
                                      name=f"bc{h}")
                        nc.gpsimd.partition_broadcast(bc, rc)
                        bcs.append(bc)
                        ctxss.append(ctxs)
                    for h in range(HPC):
                        eng = nc.gpsimd if h == 0 else nc.vector
                        eng.tensor_mul(
                            out=ctxTn[h * D:(h + 1) * D, q0:q0 + 512],
                            in0=ctxss[h], in1=bcs[h])

                # ---- schedule ------------------------------------------

                # pre-conveyor: only the K/Q chains for token chunk 0 —
                # the minimum for the first scores pair + exp. The V chain
                # and its transposes ride fills[0] (ctx trails a group, so
                # pump_to the T0 mark orders them in time). ACT moves:
                # ScalarE is idle before the exp conveyor starts.
                for op in chain_ops("wk", KTt, 0, act_bias=True):
                    op()
                for op in chain_ops("wq", QT, 0, act_bias=True):
                    op()

                # per-chunk filler queues (chunks 0-7 in token order;
                # chunk i covers tokens i*512..i*512+511). Each entry is
                # (ops, marks): marks[label] = index in ops after which
                # that chain/transpose set is fully EMITTED — used for
                # deadline pumping so a consumer is never emitted before
                # its producer (Tile deps follow program order).
                def build(parts):
                    ops, marks = [], {}
                    for label, lops in parts:
                        ops.extend(lops)
                        if label:
                            marks[label] = len(ops)
                    return [ops, marks]

                fills = [None] * 8
                fills[0] = build([
                    ("wv0", chain_ops("wv", VT, 0)),
                    ("T0", transpose_ops(0)),
                    ("wk1", chain_ops("wk", KTt, 1)),
                    ("wv1", chain_ops("wv", VT, 1)),
                    ("T1", transpose_ops(1)),
                    ("wk2", chain_ops("wk", KTt, 2)),
                    ("wv2", chain_ops("wv", VT, 2)),
                    ("T2", transpose_ops(2)),
                    ("wk3", chain_ops("wk", KTt, 3)),
                    ("wv3", chain_ops("wv", VT, 3)),
                    ("T3", transpose_ops(3)),
                    (None, chain_ops("wq", QT, 1)),
                ])
                fills[1] = build([
                    (None, chain_ops("wk", KTt, 4)),
                    (None, chain_ops("wv", VT, 4)),
                    (None, transpose_ops(4)),
                    (None, chain_ops("wq", QT, 2)),
                ])
                fills[2] = build([
                    (None, chain_ops("wk", KTt, 5)),
                    (None, chain_ops("wv", VT, 5)),
                    (None, transpose_ops(5)),
                    (None, chain_ops("wk", KTt, 6)),
                    (None, chain_ops("wq", QT, 3)),
                ])
                fills[3] = build([
                    (None, chain_ops("wv", VT, 6)),
                    (None, transpose_ops(6)),
                    (None, chain_ops("wk", KTt, 7)),
                    (None, chain_ops("wv", VT, 7)),
                    (None, transpose_ops(7)),
                    (None, chain_ops("wq", QT, 4)),
                ])
                fills[4] = build([(None, chain_ops("wq", QT, 5))])
                fills[5] = build([(None, chain_ops("wq", QT, 6))])
                fills[6] = build([(None, chain_ops("wq", QT, 7))])
                fills[7] = build([])

                pending = None  # previous chunk's deferred flush
                for ch in range(8):
                    b, qch = divmod(ch, 4)
                    q0 = ch * 512
                    fq, marks = fills[ch]
                    pumped = [0]

                    def pump(n, fq=fq, pumped=pumped):
                        for _ in range(min(n, len(fq))):
                            fq.pop(0)()
                            pumped[0] += 1

                    def pump_to(idx, pumped=pumped, pump=pump):
                        if idx is not None:
                            pump(idx - pumped[0])

                    ctx = [psC.tile([D + 1, 512], F32, tag=f"ctx{h}",
                                    name=f"ctx{h}") for h in range(HPC)]
                    eS_pend = []  # (g, eSg) waiting for their ctx
                    for g in range(8):
                        # drip-feed fillers in <=`drip`-op doses at 4
                        # points per group so the PE FIFO never carries a
                        # burst ahead of the next scores pair
                        drip = -(-(-(-len(fq) // (8 - g))) // 4)
                        eSg = esb.tile([128, 2, 1024], F8, tag="e",
                                       name="e")
                        for kc in (2 * g, 2 * g + 1):
                            pump_to(marks.get(f"wk{kc // 4}"))
                            emit_scores_exp(b, qch, kc, eSg)
                            pump(drip)
                        eS_pend.append((g, eSg))
                        if g == 0 and pending is not None:
                            # previous chunk's last ctx groups + normalize
                            # land here, AFTER this chunk's first scores
                            # pair — the exp conveyor never waits on the
                            # chunk transition
                            pb, pq0, pctx, ppend, pch = pending
                            for g0, e0 in ppend:
                                emit_ctx(pb, g0, e0, pctx)
                            emit_normalize(pq0, pctx)
                            fq.extend(outproj_ops(pch))
                            pending = None
                        # ctx trails by one group
                        while len(eS_pend) > 1:
                            g0, e0 = eS_pend.pop(0)
                            pump_to(marks.get(f"T{g0 // 2}"))
                            emit_ctx(b, g0, e0, ctx)
                            pump(drip)
                    if ch + 1 < 8:
                        pump(len(fq))
                        pending = (b, q0, ctx, list(eS_pend), ch)
                    else:
                        while eS_pend:
                            g0, e0 = eS_pend.pop(0)
                            emit_ctx(b, g0, e0, ctx)
                            pump(2)
                        pump(len(fq))
                        # tail: pipelined per-128-token normalize +
                        # out-proj; po rotates psP/psO so matmuls never
                        # wait on the previous copy's bank
                        bcs = []
                        for h in range(HPC):
                            dn = nrm.tile([1, 512], F32, tag=f"dn{h}",
                                          name=f"dn{h}")
                            # ScalarE is idle after the last exp — keep
                            # the tail's staging copies off the DVE
                            nc.scalar.copy(dn, ctx[h][D:D + 1, :])
                            rc = nrm.tile([1, 512], F32, tag=f"rc{h}",
                                          name=f"rc{h}")
                            nc.vector.reciprocal_approx_fast(rc, dn)
                            bc = nrm.tile([D, 512], F32, tag=f"bc{h}",
                                          name=f"bc{h}")
                            nc.gpsimd.partition_broadcast(bc, rc)
                            bcs.append(bc)
                        ots = {}
                        for t4 in range(4):
                            sl = slice(t4 * 128, (t4 + 1) * 128)
                            for h in range(HPC):
                                nc.vector.tensor_mul(
                                    out=ctxTn[h * D:(h + 1) * D,
                                              q0 + t4 * 128:
                                              q0 + (t4 + 1) * 128],
                                    in0=ctx[h][0:D, sl],
                                    in1=bcs[h][:, sl])
                            tok = q0 + t4 * 128
                            for nch2 in range(HPC):
                                pool = psP if nch2 == 0 else psO
                                po = pool.tile([128, 512], F32,
                                               tag="pj" if nch2 == 0
                                               else "po", name="po")
                                nc.tensor.matmul(
                                    out=po,
                                    lhsT=ctxTn[:, tok:tok + 128],
                                    rhs=wo_sb[:, nch2 * 512:
                                              (nch2 + 1) * 512],
                                    start=True, stop=True)
                                if t4 == 0:
                                    ots[nch2] = osb.tile(
                                        [128, 4, 512], BF16,
                                        tag="ot", name="ot")
                                nc.scalar.copy(
                                    out=ots[nch2][:, t4, :], in_=po)
                                # half-chunk DMAs: first 256 tokens fly
                                # while the second half is still copying
                                if t4 in (1, 3):
                                    r0 = q0 + (t4 - 1) * 128
                                    nc.sync.dma_start(
                                        out=out[r0:r0 + 256,
                                                nch2 * 512:(nch2 + 1) * 512
                                                ].rearrange(
                                                    "(t p) c -> p t c",
                                                    p=128),
                                        in_=ots[nch2][:, t4 - 1:t4 + 1, :])
    return nc


_NC_CACHE = None


def _get_nc():
    global _NC_CACHE
    if _NC_CACHE is None:
        nc = bacc.Bacc("TRN2", target_bir_lowering=False)
        build_core_program(nc)
        nc.finalize()
        _NC_CACHE = nc
    return _NC_CACHE


def make_in_maps(x, Wq, bq, Wk, bk, Wv, bv, Wo):
    bf = ml_dtypes.bfloat16
    x = np.asarray(x, np.float32).reshape(T, C)
    # [C, T] -> [128, NCH, KCH, 512]: per (partition, token-chunk) the
    # k-chunks are 8KB-contiguous, so each chunk DMA is 128 descriptors
    xT_bf = np.ascontiguousarray(
        x.T.reshape(KCH, 128, NCH, 512).transpose(1, 2, 0, 3)).astype(bf)
    iden = np.eye(128, dtype=bf)
    Wq = np.asarray(Wq, np.float32)
    Wk = np.asarray(Wk, np.float32)
    Wv = np.asarray(Wv, np.float32)
    Wo = np.asarray(Wo, np.float32)
    bq = np.asarray(bq, np.float32)
    bk = np.asarray(bk, np.float32)
    bv = np.asarray(bv, np.float32)
    def warr(w):
        # [C, DPC] -> SBUF layout [128, KCH, DPC], contiguous
        return np.ascontiguousarray(
            w.reshape(KCH, 128, DPC).transpose(1, 0, 2)).astype(bf)

    in_maps = []
    for cidx in range(8):
        hs = slice(cidx * DPC, (cidx + 1) * DPC)
        in_maps.append(dict(
            xT=xT_bf,
            wq=warr(Wq[:, hs] * 0.125),
            wk=warr(Wk[:, hs]),
            # x16: matches the 16.0 ones column of vaug; the pair cancels
            # in softmax normalization, keeping v's fp8 mantissa bits
            wv=warr(Wv[:, hs] * 16.0),
            wo=np.ascontiguousarray(Wo[hs, :]).astype(bf),
            bqkv=np.stack([bq[hs] * 0.125, bk[hs], bv[hs] * 16.0],
                          axis=1).astype(np.float32),
            iden=iden,
        ))
    return in_maps


def kernel(x, Wq, bq, Wk, bk, Wv, bv, Wo, bo, _trace=False, _trace_kwargs=None):
    in_maps = make_in_maps(x, Wq, bq, Wk, bk, Wv, bv, Wo)
    nc = _get_nc()
    res = run_bass_kernel_spmd(
        nc, in_maps, core_ids=list(range(8)),
        trace=_trace, **(_trace_kwargs or {}))
    acc = res.results[0]["out"].astype(np.float32)
    for cidx in range(1, 8):
        acc += res.results[cidx]["out"].astype(np.float32)
    acc += np.asarray(bo, np.float32)[None, :]
    out = acc.reshape(B, N, C)
    kernel.last_results = res
    return out



# revision 17
# speedup vs baseline: 1.0031x; 1.0031x over previous
"""Multi-head attention (B=2, N=2048, C=1024, H=16, D=64) on 8 TRN2 cores.

Sharding: tensor-parallel over heads — 2 heads per core. Each core computes
Q/K/V projections for its 2 heads, attention, and a partial output
projection (its heads' slice of Wo). Host sums the 8 partial outputs + bo.

v2 structure (vs v1): the attention inner loop is a conveyor paced by
ScalarE exp (~1147ns per key tile). ctx matmuls trail their exp by one
2-key-tile group so the PE FIFO never waits on ACT; projection chains,
V transposes and out-projection matmuls are spread as single-op fillers
between groups instead of bursts. x streams in token-major [128,512]
tiles so the first K/V chain starts ~3us in. Normalize reads ctx straight
from PSUM (no staging copies); out-proj results are copied to bf16 and
DMAd as bf16 partials (host accumulates in fp32).

Per-core dataflow (all matmul inputs bf16, PSUM accumulation fp32):
  xT [1024, 4096] (x transposed on host, replicated to all cores)
  QT/KT = W.T @ x.T   -> [128 (2 heads x 64), 4096]  (lhsT=W chunk, rhs=xT)
  VT likewise, then PE-transposed into v_aug [keys, 65] per head
  (65th column = ones -> softmax denominator comes out of the ctx matmul)
  S^T = K @ Q.T  -> [keys, q] in PSUM (row-tiled: both heads concurrent);
  exp on ScalarE -> bf16 SBUF
  ctx^T_aug [65, q] = v_aug.T @ expS^T  (row 64 = denominator)
  normalize: recip(row 64) on DVE, gpsimd partition_broadcast, DVE multiply
  out_partial [4096, 1024] = ctx^T.T @ Wo_slice  (bf16 out, summed on host)

The 1/sqrt(D) scale is folded into Wq/bq on the host (exact: 0.125).
"""

import numpy as np
import ml_dtypes

import concourse.bass as bass
from concourse import bacc
import concourse.tile as tile
from concourse import mybir, library_config
from concourse.bass_utils import run_bass_kernel_spmd

BF16 = mybir.dt.bfloat16
F32 = mybir.dt.float32
F8 = mybir.dt.float8e4

B, N, C = 2, 2048, 1024
H, D = 16, 64
T = B * N              # 4096 tokens
HPC = H // 8           # heads per core = 2
DPC = HPC * D          # head dims per core = 128
KCH = C // 128         # 8 contraction chunks for projections
NCH = T // 512         # 8 token chunks of 512
KT16 = N // 128        # 16 key tiles per batch


def build_core_program(nc):
    # x pre-arranged on host to [128, token-chunk, k-chunk, 512] so each
    # 512-token chunk is ONE dma with 128 contiguous-8KB descriptors
    xT = nc.dram_tensor("xT", [128, NCH, KCH, 512], BF16,
                        kind="ExternalInput").ap()
    # weights pre-arranged on host to the SBUF layout [128, KCH, DPC] so
    # their DMAs are straight 2KB-row copies (the on-the-fly rearrange
    # generated 1024 tiny 256B descriptors and took ~4us per weight)
    wq = nc.dram_tensor("wq", [128, KCH, DPC], BF16,
                        kind="ExternalInput").ap()
    wk = nc.dram_tensor("wk", [128, KCH, DPC], BF16,
                        kind="ExternalInput").ap()
    wv = nc.dram_tensor("wv", [128, KCH, DPC], BF16,
                        kind="ExternalInput").ap()
    wo = nc.dram_tensor("wo", [DPC, C], BF16, kind="ExternalInput").ap()
    bqkv = nc.dram_tensor("bqkv", [DPC, 3], F32, kind="ExternalInput").ap()
    iden = nc.dram_tensor("iden", [128, 128], BF16, kind="ExternalInput").ap()
    out = nc.dram_tensor("out", [T, C], BF16, kind="ExternalOutput").ap()

    with tile.TileContext(nc) as tc:
        with tc.tile_pool(name="singles", bufs=1) as singles:
            # DMA trigger issue costs ~620ns each on an engine queue, so
            # trigger ORDER and COUNT dominate startup. Urgent data (token
            # chunk 0 + proj weights) goes first on Sync; the bulk of x
            # rides 8 big [128,3584] DMAs issued from the ScalarE queue
            # (idle until the exp conveyor starts). load_library is
            # emitted after all input DMAs — its ~6us IRAM load otherwise
            # blocks the first triggers.
            # warmup fodder (no DMA dependency): ramps the HAM clock gate
            # from t~0 while input DMAs are still in flight, and forces
            # the Exp ACT table load off the critical path
            warm = singles.tile([128, 256], BF16, tag="warm")
            nc.vector.memset(warm, 0.015625)
            edum = singles.tile([1, 8], F32, tag="edum")
            nc.vector.memset(edum, 0.5)
            edumo = singles.tile([1, 8], BF16, tag="edumo")

            # HWDGE DMAs complete FIFO per issuing ring, so Sync-queue
            # issue ORDER is the priority: scores-path inputs first
            # (x chunk0, wk, bqkv, wq), then x chunk1/wv/iden, then bulk.
            # x chunks 0-1 live in their own tile so their consumers'
            # sem waits never reference the bulk-x DMAs.
            xu_sb = singles.tile([128, 2, KCH, 512], BF16, tag="xu")
            nc.sync.dma_start(out=xu_sb[:, 0], in_=xT[:, 0])
            w_sb = {}
            t = singles.tile([128, KCH, DPC], BF16, tag="wk", name="wk_sb")
            nc.sync.dma_start(out=t, in_=wk)
            w_sb["wk"] = [t[:, k, :] for k in range(KCH)]
            bqkv_sb = singles.tile([DPC, 3], F32, tag="bqkv")
            nc.sync.dma_start(out=bqkv_sb, in_=bqkv)
            b_sb = {"q": bqkv_sb[:, 0:1], "k": bqkv_sb[:, 1:2],
                    "v": bqkv_sb[:, 2:3]}
            t = singles.tile([128, KCH, DPC], BF16, tag="wq", name="wq_sb")
            nc.sync.dma_start(out=t, in_=wq)
            w_sb["wq"] = [t[:, k, :] for k in range(KCH)]
            nc.sync.dma_start(out=xu_sb[:, 1], in_=xT[:, 1])
            t = singles.tile([128, KCH, DPC], BF16, tag="wv", name="wv_sb")
            nc.sync.dma_start(out=t, in_=wv)
            w_sb["wv"] = [t[:, k, :] for k in range(KCH)]
            id_sb = singles.tile([128, 128], BF16, tag="iden")
            nc.sync.dma_start(out=id_sb, in_=iden)
            xb_sb = singles.tile([128, NCH - 2, KCH, 512], BF16, tag="xb")
            for c in range(2, NCH):
                nc.sync.dma_start(out=xb_sb[:, c - 2], in_=xT[:, c])
            wo_sb = singles.tile([DPC, C], BF16, tag="wo")
            nc.sync.dma_start(out=wo_sb, in_=wo)

            nc.gpsimd.load_library(library_config.proxy)

            def xslice(k, nch):
                if nch < 2:
                    return xu_sb[:, nch, k, :]
                return xb_sb[:, nch - 2, k, :]

            QT = singles.tile([128, T], BF16, tag="QT")
            KTt = singles.tile([128, T], BF16, tag="KT")
            VT = singles.tile([128, T], BF16, tag="VT")
            ctxTn = singles.tile([128, T], BF16, tag="ctxTn")
            # v_aug pairs for fp8 DoubleRow ctx: [128 keys, group, j, 80]
            # (j = which key tile of the pair; stride 80 keeps the DR
            # k-tile step 16B-aligned). col 64 = 16.0: Wv is host-scaled
            # x16 so v and the ones column carry the same factor and the
            # softmax normalization cancels it exactly.
            vaug = [[singles.tile([128, KT16 // 2, 2, 80], F8,
                                  tag=f"vaug{b}{h}", name=f"vaug{b}{h}")
                     for h in range(HPC)] for b in range(B)]
            for b in range(B):
                for h in range(HPC):
                    nc.vector.memset(vaug[b][h][:, :, :, D:D + 1], 16.0)

            with tc.tile_pool(name="psP", bufs=1, space="PSUM") as psP, \
                    tc.tile_pool(name="psO", bufs=1, space="PSUM") as psO, \
                    tc.tile_pool(name="psS", bufs=2, space="PSUM") as psS, \
                    tc.tile_pool(name="psC", bufs=1, space="PSUM") as psC, \
                    tc.tile_pool(name="esb", bufs=6) as esb, \
                    tc.tile_pool(name="nrm", bufs=3) as nrm, \
                    tc.tile_pool(name="osb", bufs=3) as osb:

                # HAM warmup on a memset tile — zero DMA dependency, so
                # the PE clock ramps from t~0 while x/weights stream in.
                # ~12x256 rows keeps PE continuously busy ~3us (the ramp
                # window). The dummy exp pulls the 1.3us Exp ACT_TABLE_LOAD
                # to t~0 instead of stalling the first real exp.
                nc.scalar.activation(edumo, edum,
                                     mybir.ActivationFunctionType.Exp)
                for wu in range(12):
                    pw = psO.tile([128, 256], F32, tag="po", name="pw")
                    nc.tensor.matmul(out=pw, lhsT=warm[:, 0:128],
                                     rhs=warm, start=True, stop=True)

                # ---- filler op factories (each closure emits ~1 PE op) --

                def chain_ops(nm, dstT, nch, act_bias=False):
                    """QKV projection chain: 8 accumulating matmuls into a
                    psP bank + one bias-add move to SBUF. Returns 9 ops."""
                    st = {}

                    def mk(k):
                        def op():
                            if k == 0:
                                st["ps"] = psP.tile([128, 512], F32,
                                                    tag="pj", name="pj")
                            nc.tensor.matmul(
                                out=st["ps"], lhsT=w_sb[nm][k],
                                rhs=xslice(k, nch),
                                start=(k == 0), stop=(k == KCH - 1))
                        return op

                    def mv():
                        dst = dstT[:, nch * 512:(nch + 1) * 512]
                        if act_bias:
                            nc.scalar.activation(
                                out=dst, in_=st["ps"],
                                func=mybir.ActivationFunctionType.Identity,
                                bias=b_sb[nm[1]], scale=1.0)
                        else:
                            nc.vector.tensor_scalar_add(
                                out=dst, in0=st["ps"], scalar1=b_sb[nm[1]])
                    return [mk(k) for k in range(KCH)] + [mv]

                def transpose_ops(nch):
                    """4 V transposes for token chunk nch -> vaug tiles."""
                    ops = []

                    def mk(t16):
                        def op():
                            b, bt = divmod(t16, KT16)
                            g, j = divmod(bt, 2)
                            pt = psO.tile([128, 128], BF16, tag="po",
                                          name="pt")
                            base = t16 * 128
                            nc.tensor.transpose(
                                pt, VT[:, base:base + 128], id_sb)
                            nc.vector.tensor_copy(
                                out=vaug[b][0][:, g, j, 0:D],
                                in_=pt[:, 0:D])
                            nc.vector.tensor_copy(
                                out=vaug[b][1][:, g, j, 0:D],
                                in_=pt[:, D:2 * D])
                        return op
                    for t16 in range(nch * 4, nch * 4 + 4):
                        ops.append(mk(t16))
                    return ops

                def outproj_ops(ch):
                    """8 out-proj matmuls for chunk ch (MM -> bf16 copy
                    into a [128,4,512] staging tile; one DMA per c-half =
                    2 Sync triggers per chunk)."""
                    q0 = ch * 512
                    st = {}
                    ops = []

                    def mk(j):
                        def op():
                            nch2, t4 = divmod(j, 4)
                            tok = q0 + t4 * 128
                            po = psO.tile([128, 512], F32, tag="po",
                                          name="po")
                            nc.tensor.matmul(
                                out=po, lhsT=ctxTn[:, tok:tok + 128],
                                rhs=wo_sb[:, nch2 * 512:(nch2 + 1) * 512],
                                start=True, stop=True)
                            if t4 == 0:
                                st[nch2] = osb.tile([128, 4, 512], BF16,
                                                    tag="ot", name="ot")
                            nc.vector.tensor_copy(out=st[nch2][:, t4, :],
                                                  in_=po)
                            if t4 == 3:
                                nc.sync.dma_start(
                                    out=out[q0:q0 + 512,
                                            nch2 * 512:(nch2 + 1) * 512
                                            ].rearrange(
                                                "(t p) c -> p t c", p=128),
                                    in_=st[nch2])
                        return op
                    for j in range(8):
                        ops.append(mk(j))
                    return ops

                # ---- attention pieces ----------------------------------

                def emit_scores_exp(b, qch, kc, eSg):
                    """Row-tiled packed scores (both heads concurrent) +
                    exp into half of the group's fp8 eS pair tile."""
                    q0 = b * N + qch * 512
                    k0 = b * N + kc * 128
                    pS = psS.tile([128, 1024], F32, tag="s", name="s")
                    for h in range(HPC):
                        nc.tensor.matmul(
                            out=pS[:, h * 512:(h + 1) * 512],
                            lhsT=KTt[h * D:(h + 1) * D, k0:k0 + 128],
                            rhs=QT[h * D:(h + 1) * D, q0:q0 + 512],
                            start=True, stop=True)
                    nc.scalar.activation(
                        eSg[:, kc % 2, :], pS,
                        mybir.ActivationFunctionType.Exp)

                def emit_ctx(b, g, eSg, ctx):
                    """One fp8 DoubleRow matmul per head contracts the
                    group's 2 key tiles (256 keys) at 0.5 cyc/row."""
                    for h in range(HPC):
                        nc.tensor.matmul(
                            out=ctx[h],
                            lhsT=vaug[b][h][:, g, :, 0:D + 1],
                            rhs=eSg[:, :, h * 512:(h + 1) * 512],
                            start=(g == 0), stop=(g == KT16 // 2 - 1),
                            perf_mode=mybir.MatmulPerfMode.DoubleRow)

                def emit_normalize(q0, ctx):
                    """denominator recip -> broadcast -> multiply into
                    ctxTn. dn/ctxs staged in SBUF (custom-DVE recip can't
                    read PSUM); the big multiply runs h0 on gpsimd, h1 on
                    DVE as in v1."""
                    bcs, ctxss = [], []
                    for h in range(HPC):
                        dn = nrm.tile([1, 512], F32, tag=f"dn{h}",
                                      name=f"dn{h}")
                        nc.vector.tensor_copy(dn, ctx[h][D:D + 1, :])
                        ctxs = nrm.tile([D, 512], F32, tag=f"ctxs{h}",
                                        name=f"ctxs{h}")
                        nc.vector.tensor_copy(ctxs, ctx[h][0:D, :])
                        rc = nrm.tile([1, 512], F32, tag=f"rc{h}",
                                      name=f"rc{h}")
                        nc.vector.reciprocal_approx_fast(rc, dn)
                        bc = nrm.tile([D, 512], F32, tag=f"bc{h}",
                                      name=f"bc{h}")
                        nc.gpsimd.partition_broadcast(bc, rc)
                        bcs.append(bc)
                        ctxss.append(ctxs)
                    for h in range(HPC):
                        eng = nc.gpsimd if h == 0 else nc.vector
                        eng.tensor_mul(
                            out=ctxTn[h * D:(h + 1) * D, q0:q0 + 512],
                            in0=ctxss[h], in1=bcs[h])

                # ---- schedule ------------------------------------------

                # pre-conveyor: only the K/Q chains for token chunk 0 —
                # the minimum for the first scores pair + exp. The V chain
                # and its transposes ride fills[0] (ctx trails a group, so
                # pump_to the T0 mark orders them in time). ACT moves:
                # ScalarE is idle before the exp conveyor starts.
                for op in chain_ops("wk", KTt, 0, act_bias=True):
                    op()
                for op in chain_ops("wq", QT, 0, act_bias=True):
                    op()

                # per-chunk filler queues (chunks 0-7 in token order;
                # chunk i covers tokens i*512..i*512+511). Each entry is
                # (ops, marks): marks[label] = index in ops after which
                # that chain/transpose set is fully EMITTED — used for
                # deadline pumping so a consumer is never emitted before
                # its producer (Tile deps follow program order).
                def build(parts):
                    ops, marks = [], {}
                    for label, lops in parts:
                        ops.extend(lops)
                        if label:
                            marks[label] = len(ops)
                    return [ops, marks]

                fills = [None] * 8
                fills[0] = build([
                    ("wv0", chain_ops("wv", VT, 0)),
                    ("T0", transpose_ops(0)),
                    ("wk1", chain_ops("wk", KTt, 1)),
                    ("wv1", chain_ops("wv", VT, 1)),
                    ("T1", transpose_ops(1)),
                    ("wk2", chain_ops("wk", KTt, 2)),
                    ("wv2", chain_ops("wv", VT, 2)),
                    ("T2", transpose_ops(2)),
                    ("wk3", chain_ops("wk", KTt, 3)),
                    ("wv3", chain_ops("wv", VT, 3)),
                    ("T3", transpose_ops(3)),
                    (None, chain_ops("wq", QT, 1)),
                ])
                fills[1] = build([
                    (None, chain_ops("wk", KTt, 4)),
                    (None, chain_ops("wv", VT, 4)),
                    (None, transpose_ops(4)),
                    (None, chain_ops("wq", QT, 2)),
                ])
                fills[2] = build([
                    (None, chain_ops("wk", KTt, 5)),
                    (None, chain_ops("wv", VT, 5)),
                    (None, transpose_ops(5)),
                    (None, chain_ops("wk", KTt, 6)),
                    (None, chain_ops("wq", QT, 3)),
                ])
                fills[3] = build([
                    (None, chain_ops("wv", VT, 6)),
                    (None, transpose_ops(6)),
                    (None, chain_ops("wk", KTt, 7)),
                    (None, chain_ops("wv", VT, 7)),
                    (None, transpose_ops(7)),
                    (None, chain_ops("wq", QT, 4)),
                ])
                fills[4] = build([(None, chain_ops("wq", QT, 5))])
                fills[5] = build([(None, chain_ops("wq", QT, 6))])
                fills[6] = build([(None, chain_ops("wq", QT, 7))])
                fills[7] = build([])

                pending = None  # previous chunk's deferred flush
                for ch in range(8):
                    b, qch = divmod(ch, 4)
                    q0 = ch * 512
                    fq, marks = fills[ch]
                    pumped = [0]

                    def pump(n, fq=fq, pumped=pumped):
                        for _ in range(min(n, len(fq))):
                            fq.pop(0)()
                            pumped[0] += 1

                    def pump_to(idx, pumped=pumped, pump=pump):
                        if idx is not None:
                            pump(idx - pumped[0])

                    ctx = [psC.tile([D + 1, 512], F32, tag=f"ctx{h}",
                                    name=f"ctx{h}") for h in range(HPC)]
                    eS_pend = []  # (g, eSg) waiting for their ctx
                    for g in range(8):
                        # drip-feed fillers in <=`drip`-op doses at 4
                        # points per group so the PE FIFO never carries a
                        # burst ahead of the next scores pair
                        drip = -(-(-(-len(fq) // (8 - g))) // 4)
                        eSg = esb.tile([128, 2, 1024], F8, tag="e",
                                       name="e")
                        for kc in (2 * g, 2 * g + 1):
                            pump_to(marks.get(f"wk{kc // 4}"))
                            emit_scores_exp(b, qch, kc, eSg)
                            pump(drip)
                        eS_pend.append((g, eSg))
                        if g == 0 and pending is not None:
                            # previous chunk's last ctx groups + normalize
                            # land here, AFTER this chunk's first scores
                            # pair — the exp conveyor never waits on the
                            # chunk transition
                            pb, pq0, pctx, ppend, pch = pending
                            for g0, e0 in ppend:
                                emit_ctx(pb, g0, e0, pctx)
                            emit_normalize(pq0, pctx)
                            fq.extend(outproj_ops(pch))
                            pending = None
                        # ctx trails by one group
                        while len(eS_pend) > 1:
                            g0, e0 = eS_pend.pop(0)
                            pump_to(marks.get(f"T{g0 // 2}"))
                            emit_ctx(b, g0, e0, ctx)
                            pump(drip)
                    if ch + 1 < 8:
                        pump(len(fq))
                        pending = (b, q0, ctx, list(eS_pend), ch)
                    else:
                        while eS_pend:
                            g0, e0 = eS_pend.pop(0)
                            emit_ctx(b, g0, e0, ctx)
                            pump(2)
                        pump(len(fq))
                        # tail: pipelined per-128-token normalize +
                        # out-proj; po rotates psP/psO so matmuls never
                        # wait on the previous copy's bank
                        bcs = []
                        for h in range(HPC):
                            dn = nrm.tile([1, 512], F32, tag=f"dn{h}",
                                          name=f"dn{h}")
                            # ScalarE is idle after the last exp — keep
                            # the tail's staging copies off the DVE
                            nc.scalar.copy(dn, ctx[h][D:D + 1, :])
                            rc = nrm.tile([1, 512], F32, tag=f"rc{h}",
                                          name=f"rc{h}")
                            nc.vector.reciprocal_approx_fast(rc, dn)
                            bc = nrm.tile([D, 512], F32, tag=f"bc{h}",
                                          name=f"bc{h}")
                            nc.gpsimd.partition_broadcast(bc, rc)
                            bcs.append(bc)
                        ots = {}
                        for t4 in range(4):
                            sl = slice(t4 * 128, (t4 + 1) * 128)
                            for h in range(HPC):
                                nc.vector.tensor_mul(
                                    out=ctxTn[h * D:(h + 1) * D,
                                              q0 + t4 * 128:
                                              q0 + (t4 + 1) * 128],
                                    in0=ctx[h][0:D, sl],
                                    in1=bcs[h][:, sl])
                            tok = q0 + t4 * 128
                            for nch2 in range(HPC):
                                pool = psP if nch2 == 0 else psO
                                po = pool.tile([128, 512], F32,
                                               tag="pj" if nch2 == 0
                                               else "po", name="po")
                                nc.tensor.matmul(
                                    out=po,
                                    lhsT=ctxTn[:, tok:tok + 128],
                                    rhs=wo_sb[:, nch2 * 512:
                                              (nch2 + 1) * 512],
                                    start=True, stop=True)
                                if t4 == 0:
                                    ots[nch2] = osb.tile(
                                        [128, 4, 512], BF16,
                                        tag="ot", name="ot")
                                nc.scalar.copy(
                                    out=ots[nch2][:, t4, :], in_=po)
                                # half-chunk DMAs: first 256 tokens fly
                                # while the second half is still copying
                                if t4 in (1, 3):
                                    r0 = q0 + (t4 - 1) * 128
                                    nc.sync.dma_start(
                                        out=out[r0:r0 + 256,
                                                nch2 * 512:(nch2 + 1) * 512
                                                ].rearrange(
                                                    "(t p) c -> p t c",
                                                    p=128),
                                        in_=ots[nch2][:, t4 - 1:t4 + 1, :])
    return nc


_NC_CACHE = None


def _get_nc():
    global _NC_CACHE
    if _NC_CACHE is None:
        nc = bacc.Bacc("TRN2", target_bir_lowering=False)
        build_core_program(nc)
        nc.finalize()
        _NC_CACHE = nc
    return _NC_CACHE


def make_in_maps(x, Wq, bq, Wk, bk, Wv, bv, Wo):
    bf = ml_dtypes.bfloat16
    x = np.asarray(x, np.float32).reshape(T, C)
    # [C, T] -> [128, NCH, KCH, 512]: per (partition, token-chunk) the
    # k-chunks are 8KB-contiguous, so each chunk DMA is 128 descriptors
    xT_bf = np.ascontiguousarray(
        x.T.reshape(KCH, 128, NCH, 512).transpose(1, 2, 0, 3)).astype(bf)
    iden = np.eye(128, dtype=bf)
    Wq = np.asarray(Wq, np.float32)
    Wk = np.asarray(Wk, np.float32)
    Wv = np.asarray(Wv, np.float32)
    Wo = np.asarray(Wo, np.float32)
    bq = np.asarray(bq, np.float32)
    bk = np.asarray(bk, np.float32)
    bv = np.asarray(bv, np.float32)
    def warr(w):
        # [C, DPC] -> SBUF layout [128, KCH, DPC], contiguous
        return np.ascontiguousarray(
            w.reshape(KCH, 128, DPC).transpose(1, 0, 2)).astype(bf)

    in_maps = []
    for cidx in range(8):
        hs = slice(cidx * DPC, (cidx + 1) * DPC)
        in_maps.append(dict(
            xT=xT_bf,
            wq=warr(Wq[:, hs] * 0.125),
            wk=warr(Wk[:, hs]),
            # x16: matches the 16.0 ones column of vaug; the pair cancels
            # in softmax normalization, keeping v's fp8 mantissa bits
            wv=warr(Wv[:, hs] * 16.0),
            wo=np.ascontiguousarray(Wo[hs, :]).astype(bf),
            bqkv=np.stack([bq[hs] * 0.125, bk[hs], bv[hs] * 16.0],
                          axis=1).astype(np.float32),
            iden=iden,
        ))
    return in_maps


def kernel(x, Wq, bq, Wk, bk, Wv, bv, Wo, bo, _trace=False, _trace_kwargs=None):
    in_maps = make_in_maps(x, Wq, bq, Wk, bk, Wv, bv, Wo)
    nc = _get_nc()
    res = run_bass_kernel_spmd(
        nc, in_maps, core_ids=list(range(8)),
        trace=_trace, **(_trace_kwargs or {}))
    acc = res.results[0]["out"].astype(np.float32)
    for cidx in range(1, 8):
        acc += res.results[cidx]["out"].astype(np.float32)
    acc += np.asarray(bo, np.float32)[None, :]
    out = acc.reshape(B, N, C)
    kernel.last_results = res
    return out

